# revision 21
# baseline (speedup 1.0000x reference)
"""Trainium2 Bass kernel for nn_AdapterMLP (gnn_message_passing).

Strategy (8 independent NeuronCores, no collectives):
  - Shard (batch=4) x (seq halves=2) -> 8 shards of [1024, 4096] rows.
  - All gather/scatter index structure is resolved on the host into
    dense one-hot matmul operands (A_g for the subtoken mean-pool
    "message passing" gather, S_sel for the last-wins scatter), so the
    device kernel is pure dense compute.
  - The host pass that casts x to fp8 also folds in the per-row
    rms-norm scale (and ln_weight folds into Wh), so the device sees
    pre-normalized activations and the epilogue scale is a constant.
  - Algebraic shortcut: aw[w,e] = <ents_t[w,e,:], b[w,:]> is computed as
    <(g*u)[w,e,:], (b @ down_w)[w,:]>, eliminating the [1152,1024]x
    [1024,4096] per-item down-projection (8x fewer word-branch FLOPs).
  - The scatter branch is folded into the big MLP as one extra K-tile:
    pre = x_n @ Wh'^T + [tmpT | 1] @ [Wt^T ; mlp_b], where
    Wh' = mlp_w[:, :D] * ln_weight.
  - The big MLP GEMM and the b@down_w GEMM run in fp8(e4m3) with
    perf_mode=DoubleRow (2 k-tiles contracted per instruction, ~1.5x
    over bf16).  Operands are pre-scaled by powers of two on the host
    (x*32, Wh'*2048) and the product scale 2^-16 is folded into the
    epilogue silu scale; the aux k-tile (scatter branch) stays bf16 and
    its weights carry 2^16 so every PSUM contribution shares one scale.
"""
import os
import sys

sys.path.insert(0, "/opt/trn_rl_repo")

import numpy as np
from ml_dtypes import bfloat16, float8_e4m3

import concourse.bass as bass
import concourse.bacc as bacc
import concourse.tile as tile
from concourse import mybir
from concourse.bass_utils import run_bass_kernel_spmd

B, S, D = 4, 2048, 4096
W, E, T = 128, 8, 4
KD, KI = 100, 1024
EPS = 1e-06
NCORES = 8
SL = S // 2        # 1024 rows per core
GR = 512           # gathered rows per core (W*T upper bound)
P = 128
FB = 512           # psum free dim
NK = D // P        # 32 k-tiles
NP = NK // 2       # 16 k-pairs (DoubleRow)
NN = D // FB       # 8 n-chunks
NM = SL // P       # 8 m-tiles
NE = E + 1         # 9

SX = 32.0          # fp8 scale on x
SWH = 2048.0       # fp8 scale on Wh'
SB = 16.0          # fp8 scale on b (folded into ag on host)
SDW = 1024.0       # fp8 scale on down_w*lnw
SMAIN = SX * SWH   # 65536 = 2^16
SAW = SB * SDW     # 16384

f32 = mybir.dt.float32
bf = mybir.dt.bfloat16
f8 = mybir.dt.float8e4
DR = mybir.MatmulPerfMode.DoubleRow
AF = mybir.ActivationFunctionType
ALU = mybir.AluOpType
AX = mybir.AxisListType


def _bf(a):
    return np.ascontiguousarray(a.astype(bfloat16))


def _f8(a, scale):
    return np.ascontiguousarray(
        np.clip(a * scale, -239.0, 239.0).astype(float8_e4m3))


def _pair_pack(kt, inner):
    """[D, inner] -> [NPAIR, P, 2*inner] with j-major pair halves."""
    d = kt.shape[0]
    np_ = d // 256
    return np.ascontiguousarray(
        kt.reshape(np_, 2, P, inner).transpose(0, 2, 1, 3).reshape(
            np_, P, 2 * inner))


def build_core_inputs(inp, core):
    """Host preprocessing for one core: slice/transpose/cast + index->matrix."""
    b, h = core // 2, core % 2
    r0 = h * SL
    x = np.asarray(inp["output_hidden_states"], np.float32)
    we_b = np.asarray(inp["words_ents"])[b]
    ws_b = np.asarray(inp["words_subtoken"])[b]
    ce = np.asarray(inp["concept_embed"], np.float32)
    sent = np.asarray(inp["sentinel"], np.float32).reshape(KD)
    lnw = np.asarray(inp["ln_weight"], np.float32)
    gw = np.asarray(inp["gate_w"], np.float32)
    uw = np.asarray(inp["up_w"], np.float32)
    dw = np.asarray(inp["down_w"], np.float32)
    mw = np.asarray(inp["mlp_w"], np.float32)
    mb = np.asarray(inp["mlp_b"], np.float32)
    alpha = float(np.asarray(inp["alpha"]).reshape(-1)[0])

    # per-row rms of the full item; the fp8 cast folds the norm in
    xi = x[b]                                                # [S, D]
    rinv = 1.0 / np.sqrt(np.mean(xi * xi, axis=1) + EPS)     # [S]

    xl = xi[r0:r0 + SL]                                      # [SL, D]
    xnt = np.ascontiguousarray((xl * rinv[r0:r0 + SL, None]).T)  # [D, SL]
    xq = _pair_pack(_f8(xnt, SX), SL)                        # [NP, P, 2*SL]
    xq = xq.reshape(4, 4, P, 2 * SL).transpose(0, 2, 1, 3).reshape(
        4, P, 4 * 2 * SL)                                    # 4 pairs per DMA

    # b-gather rows: unique subtoken indices of this item (pad index S dropped)
    idxf = np.where(ws_b == -1, S, ws_b)                     # [W,T]
    flat = idxf.reshape(-1)
    uniq = np.unique(flat[flat < S])
    gidx = np.zeros(GR, np.int64)
    gidx[:uniq.size] = uniq
    xg = xi[gidx] * rinv[gidx, None]                         # [GR, D] normed
    xgp = _pair_pack(_f8(xg, SX), D)                         # [2, P, 2*D] fp8
    cnt = np.maximum(np.sum(ws_b != -1, axis=1), 1).astype(np.float32)
    ag = np.zeros((GR, W), np.float32)
    pos = {int(s_): i for i, s_ in enumerate(uniq)}
    for w in range(W):
        for t in range(T):
            s_ = int(idxf[w, t])
            if s_ < S:
                ag[pos[s_], w] += 1.0 / cnt[w]
    ag *= SB / SX     # fp8 scale: psum b ends up carrying SB
    agp = _pair_pack(_f8(ag, 1.0), W)                        # [2, P, 2*W] fp8

    # entity embeddings (host gather of the concept table)
    we_idx = np.where(we_b == -1, 0, we_b)
    ents = ce[we_idx]                                        # [W,E,KD]
    ent_ori = np.concatenate(
        [ents, np.broadcast_to(sent.reshape(1, 1, KD), (W, 1, KD))], axis=1)
    entw = np.ascontiguousarray(ent_ori.transpose(1, 0, 2))  # [NE, W, KD]
    entt = np.zeros((P, NE * W), np.float32)                 # KD padded to 128
    entt[:KD] = entw.reshape(NE * W, KD).T

    # scatter one-hot: winner = last (w,t) in flat order; local half only
    sst = np.zeros((W, SL), np.float32)
    winner = {}
    for w in range(W):
        for t in range(T):
            s_ = int(idxf[w, t])
            if s_ < S:
                winner[s_] = w
    for s_, w in winner.items():
        if r0 <= s_ < r0 + SL:
            sst[w, s_ - r0] = 1.0

    # weights: fold ln into Wh and down_w; fp8-quantize; pair-pack
    whT = (mw[:, :D] * lnw[None, :]).T                       # [D, D]
    wq = np.zeros((NN, 4, P, 4 * 1024), float8_e4m3)         # 4 pairs per DMA
    for n in range(NN):
        pp = _pair_pack(_f8(whT[:, n * FB:(n + 1) * FB], SWH), FB)  # [NP,P,2FB]
        wq[n] = pp.reshape(4, 4, P, 2 * FB).transpose(0, 2, 1, 3).reshape(
            4, P, 4 * 1024)
    wtT = mw[:, D:].T                                        # [KD, D]
    wk_aux = np.zeros((NN, P, FB), np.float32)
    for n in range(NN):
        cs = slice(n * FB, (n + 1) * FB)
        wk_aux[n, :KD] = wtT[:, cs] * SMAIN
        wk_aux[n, KD] = mb[cs] * SMAIN
    dwt = _pair_pack(_f8((dw * lnw[:, None]), SDW), KI)      # [NP, P, 2*KI]
    dwt = dwt.reshape(2, 8, P, 2 * KI).transpose(0, 2, 1, 3).reshape(
        2, P, 16 * KI)                                       # 8 pairs per DMA

    mask = np.where(
        np.concatenate([we_b, np.ones((W, 1), we_b.dtype)], -1) == -1,
        -1e9, 0.0).astype(np.float32)

    aux_init = np.zeros((P, SL), np.float32)
    aux_init[KD] = 1.0

    return dict(
        xq=xq,
        xrow=np.ascontiguousarray(xl),
        wq=wq,
        wk_aux=_bf(wk_aux),
        dwt=dwt,
        xg=xgp,
        ag=agp,
        entw=_bf(entw),
        entt=_bf(entt),
        gwt=_bf(np.concatenate([gw.T, np.zeros((P - KD, KI), np.float32)], 0)),
        uwt=_bf(np.concatenate([uw.T, np.zeros((P - KD, KI), np.float32)], 0)),
        sst=_bf(sst),
        mask=np.ascontiguousarray(mask),
        alpha_b=np.full((P, 1), alpha, np.float32),
        aux_init=_bf(aux_init),
    )


def _kernel_body(nc, tc, I, out_ap):
    with tc.tile_pool(name="res", bufs=1) as res, \
         tc.tile_pool(name="small", bufs=1) as small:
        with tc.tile_pool(name="wk0p", bufs=1) as wk0p, \
             tc.tile_pool(name="mpsum", bufs=1, space="PSUM") as mps:
            wp = tc.alloc_tile_pool(name="word", bufs=1)
            # ======== sync-queue DMAs in priority order ========
            # few, large DMAs: ~9 shared issue slots serialize in queue
            # order, so the critical tensors must be the first transfers
            xg_tiles = []
            for q in range(2):
                xg_t = wp.tile([P, 2 * D], f8, tag=f"xg{q}", name=f"xg{q}")
                nc.sync.dma_start(out=xg_t[:, 0:D], in_=I["xg"][q][:, 0:D])
                nc.sync.dma_start(out=xg_t[:, D:2 * D],
                                  in_=I["xg"][q][:, D:2 * D])
                xg_tiles.append(xg_t)
            # dwt next (needed by the c-matmul right after bT)
            dwt_tiles = []
            for q in range(2):
                dwt_t = wp.tile([P, 16 * KI], f8, tag=f"dwt{q}",
                                name=f"dwt{q}")
                nc.sync.dma_start(out=dwt_t[:], in_=I["dwt"][q])
                dwt_tiles.append(dwt_t)

            def dwt_view(kb, i):
                return dwt_tiles[kb // 8][:].rearrange(
                    "p (q j c) -> p q j c", q=8, j=2)[
                        :, kb % 8, :, i * FB:(i + 1) * FB]

            xq_tiles = []
            for g in range(4):
                t = res.tile([P, 8 * SL], f8, tag=f"xq{g}", name=f"xq{g}")
                nc.sync.dma_start(out=t[:], in_=I["xq"][g])
                xq_tiles.append(t)

            def xq_view(kp, m):
                return xq_tiles[kp // 4][:].rearrange(
                    "p (q j s) -> p q j s", q=4, j=2)[
                        :, kp % 4, :, m * P:(m + 1) * P]

            # small/late word inputs trail on the sync queue
            aux_t = res.tile([P, SL], bf, tag="aux")
            nc.sync.dma_start(out=aux_t[:], in_=I["aux_init"][:])
            alpha_t = small.tile([P, 1], f32, tag="alpha")
            nc.sync.dma_start(out=alpha_t[:], in_=I["alpha_b"][:])
            mask_t = small.tile([P, NE], f32, tag="mask")
            nc.sync.dma_start(out=mask_t[:], in_=I["mask"][:])
            sst_t = wp.tile([P, SL], bf, tag="sst")
            nc.sync.dma_start(out=sst_t[:], in_=I["sst"][:])
            ent_t = wp.tile([P, NE * KD], bf, tag="entw")
            for e in range(NE):
                nc.sync.dma_start(out=ent_t[:, e * KD:(e + 1) * KD],
                                  in_=I["entw"][e])

            # ======== gpsimd queue: word smalls, then wq chunk 0 ========
            # (gpsimd has its own DMA slot pool; keeps ACT sequencer free)
            agm = wp.tile([P, 4 * W], f8, tag="agm")
            for q in range(2):
                nc.gpsimd.dma_start(out=agm[:, q * 2 * W:(q + 1) * 2 * W],
                                    in_=I["ag"][q])
            entt_t = wp.tile([P, NE * W], bf, tag="entt")
            nc.gpsimd.dma_start(out=entt_t[:], in_=I["entt"][:])
            gwt_t = wp.tile([P, KI], bf, tag="gwt")
            nc.gpsimd.dma_start(out=gwt_t[:], in_=I["gwt"][:])
            uwt_t = wp.tile([P, KI], bf, tag="uwt")
            nc.gpsimd.dma_start(out=uwt_t[:], in_=I["uwt"][:])

            wk_cache = {}
            grp0 = []
            for j in range(4):
                wt = wk0p.tile([P, 4 * 1024], f8, tag=f"wk0g{j}",
                               name=f"wk0g{j}")
                nc.gpsimd.dma_start(out=wt[:], in_=I["wq"][0, j])
                grp0.append(wt)
            at0 = wk0p.tile([P, FB], bf, tag="wk0aux")
            nc.gpsimd.dma_start(out=at0[:], in_=I["wk_aux"][0])
            wk_cache[0] = (grp0, at0)

            def wq_view(grp, kp):
                return grp[kp // 4][:].rearrange(
                    "p (q j c) -> p q j c", q=4, j=2)[:, kp % 4]

            # ======== word branch compute ========
            # PE is a strict FIFO: emit self-paced matmul chains (bT, c)
            # first, then the ACT/DVE-paced gate/up blocks interleaved
            # with main-GEMM filler MMs so the PE never blocks.
            pms03 = []
            fill = {"m": 0, "kp": 0}

            def emit_fill(nmm, tag="pm"):
                for _ in range(nmm):
                    m, kp = fill["m"], fill["kp"]
                    if m >= 6:
                        return
                    if kp == 0:
                        pm = mps.tile([P, FB], f32, tag=tag,
                                      bufs=4 if tag == "pm" else 2,
                                      name=f"pm0_{m}")
                        pms03.append(pm)
                    nc.tensor.matmul(
                        pms03[m][:], lhsT=xq_view(kp, m),
                        rhs=wq_view(grp0, kp),
                        start=(kp == 0), stop=False, perf_mode=DR)
                    kp += 1
                    if kp == NP:
                        fill["m"], fill["kp"] = m + 1, 0
                    else:
                        fill["kp"] = kp

            with tc.tile_pool(name="wpsum", bufs=2, space="PSUM") as wps:
                # gate/up e-loop with bT blocks as the PE filler; both
                # are early-input chains (xg/ag/entt/gwt/uwt), and bT's
                # self-paced MMs cover the ACT/DVE drain stalls.
                bt_pairs = [wp.tile([P, 2 * W], f8, tag=f"btp{kb}",
                                    name=f"btp{kb}") for kb in range(NP)]
                ag_views = [
                    agm[:, q * 2 * W:(q + 1) * 2 * W].rearrange(
                        "p (j w) -> p j w", j=2) for q in range(2)]

                def emit_bt(dk):
                    ps = wps.tile([P, W], f32, tag="wps", name=f"btps{dk}")
                    for q in range(2):
                        nc.tensor.matmul(
                            ps[:],
                            lhsT=xg_tiles[q][:].rearrange(
                                "p (j d) -> p j d", j=2)[
                                    :, :, dk * P:(dk + 1) * P],
                            rhs=ag_views[q],
                            start=(q == 0), stop=(q == 1), perf_mode=DR)
                    nc.scalar.copy(
                        bt_pairs[dk // 2][:, (dk % 2) * W:(dk % 2 + 1) * W],
                        ps[:])

                gu_tiles = []
                dk_next = 0
                for e in range(NE):
                    g_sb = wp.tile([P, KI], bf, tag="gsb", bufs=2,
                                   name=f"gsb{e}")
                    gu = wp.tile([P, KI], bf, tag=f"gu{e}", name=f"gu{e}")
                    gps = []
                    for i in range(2):
                        gp = wps.tile([P, FB], f32, tag="wps", name=f"gp{e}_{i}")
                        nc.tensor.matmul(gp[:], lhsT=entt_t[:, e * P:(e + 1) * P],
                                         rhs=gwt_t[:, i * FB:(i + 1) * FB],
                                         start=True, stop=True)
                        gps.append(gp)
                    for i in range(2):
                        nc.scalar.activation(g_sb[:, i * FB:(i + 1) * FB],
                                             gps[i][:], AF.Silu)
                    ndk = 4 if e < 5 else 3
                    for dk in range(dk_next, dk_next + ndk):
                        emit_bt(dk)
                    dk_next += ndk
                    for i in range(2):
                        up = wps.tile([P, FB], f32, tag="wps", name=f"up{e}_{i}")
                        nc.tensor.matmul(up[:], lhsT=entt_t[:, e * P:(e + 1) * P],
                                         rhs=uwt_t[:, i * FB:(i + 1) * FB],
                                         start=True, stop=True)
                        nc.vector.tensor_mul(gu[:, i * FB:(i + 1) * FB],
                                             g_sb[:, i * FB:(i + 1) * FB],
                                             up[:])
                    gu_tiles.append(gu)
                assert dk_next == NK

                # PE: c = (SB*b) @ (SDW*down_w*lnw), fp8 DoubleRow
                c_bf = wp.tile([P, KI], bf, tag="c")
                cps = []
                for i in range(2):
                    cpsi = wps.tile([P, FB], f32, tag="wps", name=f"c_ps{i}")
                    cps.append(cpsi)
                for kb in range(NP):
                    lv = bt_pairs[kb][:].rearrange("p (j w) -> p j w", j=2)
                    for i in range(2):
                        nc.tensor.matmul(
                            cps[i][:], lhsT=lv,
                            rhs=dwt_view(kb, i),
                            start=(kb == 0), stop=(kb == NP - 1),
                            perf_mode=DR)
                for i in range(2):
                    nc.scalar.copy(c_bf[:, i * FB:(i + 1) * FB], cps[i][:])

                # runway: 6 main tiles' k-MMs keep the PE busy while the
                # DVE runs the aw chain below
                while fill["m"] < 4:
                    emit_fill(NP)
                emit_fill(2 * NP, tag="pm2")

                # aw[e] = <gu[e], c> via stt-accum on DVE
                aw_t = small.tile([P, NE], f32, tag="aw")
                for e in range(NE):
                    scr = wp.tile([P, KI], bf, tag="awscr", bufs=1)
                    nc.vector.scalar_tensor_tensor(
                        out=scr[:], in0=gu_tiles[e][:], scalar=1.0, in1=c_bf[:],
                        op0=ALU.mult, op1=ALU.mult,
                        accum_out=aw_t[:, e:e + 1])

                if True:
                    if True:
                        # DVE: softmax + attn chain (aw carries SB*SDW scale)
                        awm = small.tile([P, NE], f32, tag="awm")
                        nc.vector.scalar_tensor_tensor(
                            out=awm[:], in0=aw_t[:], scalar=1.0 / SAW,
                            in1=mask_t[:], op0=ALU.mult, op1=ALU.add)
                        mx = small.tile([P, 1], f32, tag="mx")
                        nc.vector.reduce_max(mx[:], awm[:], axis=AX.X)
                        nmx = small.tile([P, 1], f32, tag="nmx")
                        nc.vector.tensor_scalar_mul(nmx[:], mx[:], -1.0)
                        expt = small.tile([P, NE], f32, tag="expt")
                        sume = small.tile([P, 1], f32, tag="sume")
                        nc.scalar.activation(expt[:], awm[:], AF.Exp, bias=nmx[:],
                                             accum_out=sume[:])
                        rse = small.tile([P, 1], f32, tag="rse")
                        nc.vector.reciprocal(rse[:], sume[:])
                        attn = small.tile([P, NE], f32, tag="attn")
                        nc.vector.tensor_scalar_mul(attn[:], expt[:], rse[:])
                        acc_prev = wp.tile([P, KD], f32, tag="acc", bufs=2)
                        nc.vector.tensor_scalar_mul(acc_prev[:], ent_t[:, 0:KD],
                                                    attn[:, 0:1])
                        for e in range(1, NE):
                            acc_new = wp.tile([P, KD], f32, tag="acc", bufs=2,
                                              name=f"acc{e}")
                            nc.vector.scalar_tensor_tensor(
                                out=acc_new[:], in0=ent_t[:, e * KD:(e + 1) * KD],
                                scalar=attn[:, e:e + 1], in1=acc_prev[:],
                                op0=ALU.mult, op1=ALU.add)
                            acc_prev = acc_new
                        ao_pad = wp.tile([P, P], bf, tag="ao_pad")
                        nc.vector.memset(ao_pad[:], 0.0)
                        nc.scalar.copy(ao_pad[:, 0:KD], acc_prev[:])

                        # PE: scatter matmul into aux k-tile (gps tag --
                        # wps banks are held by runway tiles until the
                        # epilogue, which itself waits on aux)
                        for i in range(SL // FB):
                            tps = wps.tile([P, FB], f32, tag="wps", name=f"tps{i}")
                            nc.tensor.matmul(tps[:], lhsT=ao_pad[:],
                                             rhs=sst_t[:, i * FB:(i + 1) * FB],
                                             start=True, stop=True)
                            nc.scalar.copy(aux_t[0:KD, i * FB:(i + 1) * FB],
                                           tps[0:KD, :])

            if os.environ.get("K_PROBE"):
                dbg_aux = nc.dram_tensor("dbg_aux", [P, SL], bf, kind="Internal").ap()
                nc.sync.dma_start(out=dbg_aux[:], in_=aux_t[:])
                dbg_aw = nc.dram_tensor("dbg_aw", [P, NE], f32, kind="Internal").ap()
                nc.sync.dma_start(out=dbg_aw[:], in_=aw_t[:])
                dbg_c = nc.dram_tensor("dbg_c", [P, KI], bf, kind="Internal").ap()
                nc.sync.dma_start(out=dbg_c[:], in_=c_bf[:])

            # ---- word pool closed; main loop ----
            wp.release()
            with tc.tile_pool(name="op", bufs=2) as op:
                with tc.tile_pool(name="wkp", bufs=1) as wkp:
                    def wk_chunk(n):
                        if n in wk_cache:
                            return wk_cache[n]
                        grp = []
                        for j in range(4):
                            wt = wkp.tile([P, 4 * 1024], f8, tag=f"wkg{j}",
                                          bufs=2, name=f"wk{n}g{j}")
                            nc.gpsimd.dma_start(out=wt[:], in_=I["wq"][n, j])
                            grp.append(wt)
                        at = wkp.tile([P, FB], bf, tag="wk_aux", bufs=2,
                                      name=f"wka{n}")
                        nc.gpsimd.dma_start(out=at[:], in_=I["wk_aux"][n])
                        wk_cache[n] = (grp, at)
                        return wk_cache[n]

                    wk_chunk(1)
                    wk_chunk(2)
                    for n in range(NN):
                        wk_grp, wk_aux_t = wk_chunk(n)
                        for m in range(NM):
                            if n == 0 and m < 6:
                                pm = pms03[m]
                            else:
                                pm = mps.tile([P, FB], f32, tag="pm",
                                              bufs=4, name=f"pm{n}_{m}")
                                for kp in range(NP):
                                    nc.tensor.matmul(
                                        pm[:], lhsT=xq_view(kp, m),
                                        rhs=wq_view(wk_grp, kp),
                                        start=(kp == 0), stop=False,
                                        perf_mode=DR)
                            nc.tensor.matmul(pm[:], lhsT=aux_t[:, m * P:(m + 1) * P],
                                             rhs=wk_aux_t[:], start=False, stop=True)
                            pre_sb = op.tile([P, FB], f32, tag="pre", bufs=3,
                                             name=f"pre{n}_{m}")
                            nc.scalar.activation(pre_sb[:], pm[:], AF.Silu,
                                                 scale=1.0 / SMAIN)
                            xr_c = op.tile([P, FB], f32, tag="xrc", bufs=3,
                                           name=f"xrc{n}_{m}")
                            nc.sync.dma_start(
                                out=xr_c[:],
                                in_=I["xrow"][m * P:(m + 1) * P, n * FB:(n + 1) * FB])
                            nc.vector.scalar_tensor_tensor(
                                out=xr_c[:], in0=pre_sb[:], scalar=alpha_t[:],
                                in1=xr_c[:], op0=ALU.mult, op1=ALU.add)
                            nc.sync.dma_start(
                                out=out_ap[m * P:(m + 1) * P, n * FB:(n + 1) * FB],
                                in_=xr_c[:])
                        if n + 3 < NN:
                            wk_chunk(n + 3)


_CACHE = {}


def _build():
    if "nc" in _CACHE:
        return _CACHE["nc"]
    nc = bacc.Bacc("TRN2", target_bir_lowering=False, debug=False,
                   num_devices=NCORES)
    shapes = dict(
        xq=([4, P, 8 * SL], f8), xrow=([SL, D], f32),
        wq=([NN, 4, P, 4 * 1024], f8), wk_aux=([NN, P, FB], bf),
        dwt=([2, P, 16 * KI], f8), xg=([2, P, 2 * D], f8), ag=([2, P, 2 * W], f8),
        entw=([NE, W, KD], bf), entt=([P, NE * W], bf), gwt=([P, KI], bf),
        uwt=([P, KI], bf), sst=([W, SL], bf), mask=([W, NE], f32),
        alpha_b=([P, 1], f32), aux_init=([P, SL], bf),
    )
    I = {name: nc.dram_tensor(name, shp, dt, kind="ExternalInput").ap()
         for name, (shp, dt) in shapes.items()}
    out_ap = nc.dram_tensor("out", [SL, D], f32, kind="ExternalOutput").ap()
    with tile.TileContext(nc) as tc:
        _kernel_body(nc, tc, I, out_ap)
    nc.compile()
    _CACHE["nc"] = nc
    return nc


def kernel(**inputs):
    nc = _build()
    in_maps = [build_core_inputs(inputs, c) for c in range(NCORES)]
    res = run_bass_kernel_spmd(nc, in_maps, core_ids=list(range(NCORES)))
    out = np.empty((B, S, D), np.float32)
    for c in range(NCORES):
        b, h = c // 2, c % 2
        out[b, h * SL:(h + 1) * SL] = res.results[c]["out"]
    return out


if __name__ == "__main__":
    rng = np.random.default_rng(0)
    inp = {
        "output_hidden_states": rng.standard_normal((B, S, D)).astype(np.float32),
        "words_ents": rng.integers(0, 100000, (B, W, E)).astype(np.int64),
        "words_subtoken": rng.integers(0, S, (B, W, T)).astype(np.int64),
        "input_ids": rng.integers(0, 32000, (B, S)).astype(np.int64),
        "concept_embed": (rng.standard_normal((100000, KD)) * 0.02).astype(np.float32),
        "sentinel": (rng.standard_normal((1, KD)) * 0.02).astype(np.float32),
        "ln_weight": np.ones(D, np.float32),
        "gate_w": (rng.standard_normal((KI, KD)) * 0.02).astype(np.float32),
        "up_w": (rng.standard_normal((KI, KD)) * 0.02).astype(np.float32),
        "down_w": (rng.standard_normal((D, KI)) * 0.02).astype(np.float32),
        "mlp_w": (rng.standard_normal((D, D + KD)) * 0.01).astype(np.float32),
        "mlp_b": np.zeros(D, np.float32),
        "alpha": np.array([0.5], np.float32),
    }
    out = kernel(**inp)
    print("kernel ran, out shape", out.shape, "mean", out.mean())
